# revision 40
# baseline (speedup 1.0000x reference)
"""Trainium2 Bass kernel for CSAM channel self-attention module.

Per batch b (one per NeuronCore, B=8 over 8 cores):
    v      = x2[b].reshape(7, D)                 # D = 64*128*128 = 1048576
    E      = v @ v.T                             # [7,7] gram ("energy")
    att    = softmax(rowmax(E) - E, axis=-1)     # == exp(rowmin(E)-E)/Z
    out    = att @ v
    y[b]   = x1[b] * (gamma*out) + x1[b] = x1[b] * (gamma*out + 1)

Layout: d = q*65536 + w*2048 + f  (Q=16 runs, tiles [112, 2048] with
partition p = 7*q + m and 8KB contiguous DRAM lines).

Pass A: stream x2 via gpsimd SWDGE cast-DMAs (fp32 HBM -> fp16 SBUF cache
xh[w], one 917KB transfer per tile, no staging or DVE casts).  PE-transpose
[112,128] chunks -> PSUM [128,112] fp16, copy to SBUF (DVE/ACT alternating),
gram-matmul accumulate into E_psum[112,112] (diag 7x7 blocks = per-q partial
gram).  Gram matmuls trail the transposes by a few chunks so the in-order PE
queue never stalls on the copy round-trip.  fp16 is safe: top-2 energy gaps
are >100 while fp16 gram error is ~+-2.

Middle (all on-chip, ~2us): e_m = E_psum . blockdiag-mask (DVE), then
E7 = R1^T e_m R1 via two tiny PE matmuls (R1 = 16-stacked I7 folds the 16
diagonal blocks; cross-q junk is masked off first).  Softmax on-chip, then
W = mask16 . (R1x^T (a16 R1y)) -- two more tiny PE matmuls replicate
gamma*att into all 16 diagonal blocks, DVE mask keeps only those.  The
R1/R1x/R1y/mask constants are built once at startup via a DRAM bounce of I7
(engine ops cannot start at partition offsets like 7q or 112 -- only DMA can
touch those).  x1 prefetch (5 tiles on the idle HWDGE queues) covers the
middle's DMA gap.

Pass B: out_psum[112,512] = W.T @ Xh slices (fp16, "+1" fused via ones row
112, written by tiny gpsimd DMAs interleaved into the cast stream), y =
out_psum * x1 on DVE into a separate y-staging tile (an in-place variant
couples the x1-load to the y-store of the tile 8 back and strangles the
pipeline ramp -- measured 40us slower).  x2 is read from HBM exactly once.
"""

import sys

import numpy as np

try:
    import concourse.bass as bass
except ImportError:  # grading env fallback
    sys.path.insert(0, "/opt/trn_rl_repo")
    import concourse.bass as bass

from contextlib import ExitStack

import concourse.bacc as bacc
import concourse.tile as tile
from concourse import mybir
from concourse.bass_utils import run_bass_kernel_spmd
from concourse.masks import make_identity

F32 = mybir.dt.float32
F16 = mybir.dt.float16

B = 8
NN = 7              # attention dim
Q = 16              # d-runs per channel
P = NN * Q          # 112 partitions of (q, m)
PK = P + 1          # PSUM op-tile partition headroom (tile shape reuse)
FS = 2048           # stream tile free dim (8KB DRAM lines)
FM = 512            # matmul slice free dim (one PSUM bank)
D_FULL = 64 * 128 * 128
N_CORES = 8
PIPE = 8            # gram matmul trails transposes by this many chunks
GRP = 4             # transpose chunks batched per PSUM bank
X1PRE = 6           # x1 prefetch depth


def build_nc(d_total=D_FULL):
    assert d_total % (Q * FS) == 0
    ws = d_total // (Q * FS)          # stream tiles (32 at full size)
    cpt = FS // 128                   # transpose chunks per stream tile (16)
    mpt = FS // FM                    # matmul slices per stream tile (4)
    n_gram = ws * cpt

    nc = bacc.Bacc("TRN2", target_bir_lowering=False, debug=False)
    x1 = nc.dram_tensor("x1", [NN, d_total], F32, kind="ExternalInput")
    x2 = nc.dram_tensor("x2", [NN, d_total], F32, kind="ExternalInput")
    gm = nc.dram_tensor("gamma", [1], F32, kind="ExternalInput")
    y = nc.dram_tensor("y", [NN, d_total], F32, kind="ExternalOutput")

    x2v = x2[:].rearrange("m (q w f) -> q m w f", q=Q, w=ws, f=FS)
    x1v = x1[:].rearrange("m (q w f) -> q m w f", q=Q, w=ws, f=FS)
    yv = y[:].rearrange("m (q w f) -> q m w f", q=Q, w=ws, f=FS)

    with tile.TileContext(nc) as tc, ExitStack() as ctx:
        consts = ctx.enter_context(tc.tile_pool(name="consts", bufs=1))
        cache = ctx.enter_context(tc.tile_pool(name="cache", bufs=1))
        x1s = ctx.enter_context(tc.tile_pool(name="x1s", bufs=X1PRE))
        tsb = ctx.enter_context(tc.tile_pool(name="tsb", bufs=3))
        ys = ctx.enter_context(tc.tile_pool(name="ys", bufs=3))
        small = ctx.enter_context(tc.tile_pool(name="small", bufs=1))
        dramp = ctx.enter_context(tc.tile_pool(name="dramp", bufs=1, space="DRAM"))
        tps = ctx.enter_context(tc.tile_pool(name="tps", bufs=4, space="PSUM"))
        eps = ctx.enter_context(tc.tile_pool(name="eps", bufs=1, space="PSUM"))
        ops = ctx.enter_context(tc.tile_pool(name="ops", bufs=3, space="PSUM"))

        xh = [cache.tile([PK, FS], F16, name=f"xh{w}", tag=f"xh{w}")
              for w in range(ws)]
        wt = small.tile([PK, P], F16)

        # The fused-"+1" ones rows (row 112 of each xh tile and of wt) are
        # written by tiny gpsimd DMAs from a DRAM ones scratch, interleaved
        # into the cast-DMA stream AFTER each tile's cast (ranges are
        # disjoint, so order doesn't matter for the data -- but ones-first
        # made every cast wait a ones semaphore, denting the stream).  (An
        # engine memset cannot start at partition 112, and a [*,2048] memset
        # costs 1.8us of free-dim-serial engine time regardless of partition
        # count -- 32 of them at the DVE queue head stalled the whole pass.)
        ones16 = consts.tile([1, 256], F16)
        nc.vector.memset(ones16[:], 1.0)
        onescr = dramp.tile([1, FS], F16)
        nc.sync.dma_start(
            out=onescr[:],
            in_=bass.AP(tensor=ones16.tensor, offset=ones16.offset,
                        ap=[[ones16.ap[0][0], 1], [0, FS // 256], [1, 256]]),
        )

        # ---- gpsimd queue head: gamma, then the x2 cast stream ------------
        gsb = small.tile([NN, 1], F32)
        nc.gpsimd.dma_start(
            out=gsb[:],
            in_=bass.AP(tensor=gm[:].tensor, offset=0, ap=[[0, NN], [1, 1]]),
        )
        for w in range(2):
            nc.gpsimd.dma_start(out=xh[w][0:P, :], in_=x2v[:, :, w, :])

        ident = consts.tile([P, P], F16)
        make_identity(nc, ident)                       # gpsimd memset+select

        for w in range(2, ws):
            nc.gpsimd.dma_start(out=xh[w][0:P, :], in_=x2v[:, :, w, :])
            nc.gpsimd.dma_start(out=xh[w - 2][P:PK, :], in_=onescr[:])
        for w in range(ws - 2, ws):
            nc.gpsimd.dma_start(out=xh[w][P:PK, :], in_=onescr[:])
        nc.gpsimd.dma_start(out=wt[P:PK, :], in_=onescr[0:1, 0:P])

        # ---- remaining constants (off critical path) ----------------------

        # Fold/replicate operators from a DRAM bounce of I7:
        #   R1  [112,7]: R1[7q+m, n] = I7[m, n]
        #   R1r [7,112]: R1r[n, 7q+n'] = I7[n, n']
        id7_32 = consts.tile([NN, NN], F32)
        make_identity(nc, id7_32)
        id7_16 = consts.tile([NN, NN], F16)
        nc.vector.tensor_copy(out=id7_16[:], in_=id7_32[:])
        dscr = dramp.tile([NN, NN], F32)
        dscr16 = dramp.tile([NN, NN], F16)
        nc.scalar.dma_start(out=dscr[:], in_=id7_32[:])
        nc.scalar.dma_start(out=dscr16[:], in_=id7_16[:])
        r1_32 = consts.tile([P, NN], F32)
        nc.scalar.dma_start(
            out=r1_32[:],
            in_=bass.AP(tensor=dscr.tensor, offset=dscr.offset,
                        ap=[[0, Q], [NN, NN], [1, NN]]),
        )
        r1r_16 = consts.tile([NN, P], F16)
        r1r_dst = bass.AP(tensor=r1r_16.tensor, offset=r1r_16.offset,
                          ap=[[r1r_16.ap[0][0], NN], [NN, Q], [1, NN]])
        nc.scalar.dma_start(
            out=r1r_dst,
            in_=bass.AP(tensor=dscr16.tensor, offset=dscr16.offset,
                        ap=[[NN, NN], [0, Q], [1, NN]]),
        )
        # block-diag masks (built with tiny DMAs: engine writes cannot start
        # at partition 7q)
        ones7_32 = consts.tile([NN, NN], F32)
        nc.vector.memset(ones7_32[:], 1.0)
        ones7_16 = consts.tile([NN, NN], F16)
        nc.vector.memset(ones7_16[:], 1.0)
        mask32 = consts.tile([P, P], F32)
        nc.vector.memset(mask32[:], 0.0)
        mask16 = consts.tile([P, P], F16)
        nc.vector.memset(mask16[:], 0.0)
        for q in range(Q):
            s = slice(q * NN, (q + 1) * NN)
            nc.sync.dma_start(out=mask32[s, s], in_=ones7_32[:])
            nc.scalar.dma_start(out=mask16[s, s], in_=ones7_16[:])

        E = eps.tile([P, P], F32)

        # ~4us of dummy matmuls so the PE HAM clock-gate opens before the
        # real pass-A stream arrives (and stays open)
        for _ in range(24):
            wm = ops.tile([PK, FM], F32, tag="op")
            nc.tensor.matmul(wm[0:P, 0:P], lhsT=ident[:], rhs=ident[:],
                             start=True, stop=True)

        # ---------------- pass A: transpose + gram on the fp16 cache --------
        pend = []          # tt chunk APs awaiting gram matmul
        gi = 0             # gram matmuls emitted

        def emit_gram(tt_ap):
            nonlocal gi
            nc.tensor.matmul(E[:], lhsT=tt_ap, rhs=tt_ap,
                             start=(gi == 0), stop=(gi == n_gram - 1))
            gi += 1

        for w in range(ws):
            for g in range(cpt // GRP):
                tp = tps.tile([128, GRP * P], F16)
                for k in range(GRP):
                    c = g * GRP + k
                    nc.tensor.transpose(
                        tp[:, k * P:(k + 1) * P],
                        xh[w][0:P, c * 128:(c + 1) * 128], ident[:])
                tt = tsb.tile([128, GRP * P], F16)
                if (w * (cpt // GRP) + g) % 4 == 0:
                    nc.scalar.copy(tt[:], tp[:])
                else:
                    nc.vector.tensor_copy(out=tt[:], in_=tp[:])
                for k in range(GRP):
                    pend.append(tt[:, k * P:(k + 1) * P])
                while len(pend) > PIPE:
                    emit_gram(pend.pop(0))
        for tt in pend:
            emit_gram(tt)
        pend = []

        # ---------------- energy -> attention -> weights (on-chip) ---------
        e_m = small.tile([P, P], F32)
        nc.vector.tensor_mul(e_m[:], E[:], mask32[:])  # PSUM read + mask
        t1p = ops.tile([PK, FM], F32, tag="op")
        nc.tensor.matmul(t1p[0:P, 0:NN], lhsT=e_m[:], rhs=r1_32[:],
                         start=True, stop=True)        # fold n over q
        t1 = small.tile([P, NN], F32)
        nc.scalar.copy(t1[:], t1p[0:P, 0:NN])
        e7p = ops.tile([PK, FM], F32, tag="op")
        nc.tensor.matmul(e7p[0:NN, 0:NN], lhsT=r1_32[:], rhs=t1[:],
                         start=True, stop=True)        # fold m over q
        e7 = small.tile([NN, NN], F32)
        nc.vector.tensor_copy(out=e7[:], in_=e7p[0:NN, 0:NN])
        mn = small.tile([NN, 1], F32)
        nc.vector.tensor_reduce(
            out=mn[:], in_=e7[:], axis=mybir.AxisListType.X,
            op=mybir.AluOpType.min,
        )
        ex = small.tile([NN, NN], F32)
        nc.scalar.activation(
            out=ex[:], in_=e7[:], func=mybir.ActivationFunctionType.Exp,
            bias=mn[:], scale=-1.0,
        )                                              # exp(rowmin - E)
        z = small.tile([NN, 1], F32)
        nc.vector.tensor_reduce(
            out=z[:], in_=ex[:], axis=mybir.AxisListType.X,
            op=mybir.AluOpType.add,
        )
        r = small.tile([NN, 1], F32)
        nc.vector.reciprocal(r[:], z[:])
        rg = small.tile([NN, 1], F32)
        nc.vector.tensor_mul(rg[:], r[:], gsb[:])      # gamma / Z_n
        a16 = small.tile([NN, NN], F16)
        nc.vector.tensor_scalar_mul(a16[:], ex[:], rg[:])   # gamma*att, fp16
        arp = ops.tile([PK, FM], F32, tag="op")
        nc.tensor.matmul(arp[0:NN, 0:P], lhsT=a16[:], rhs=r1r_16[:],
                         start=True, stop=True)        # (g*att)^T tiled 16x
        arep = small.tile([NN, P], F16)
        nc.scalar.copy(arep[:], arp[0:NN, 0:P])
        wp = ops.tile([PK, FM], F32, tag="op")
        nc.tensor.matmul(wp[0:P, 0:P], lhsT=r1r_16[:], rhs=arep[:],
                         start=True, stop=True)        # replicate over q rows
        w_sb = small.tile([P, P], F16)
        nc.scalar.copy(w_sb[:], wp[0:P, 0:P])
        nc.vector.tensor_mul(wt[0:P, :], w_sb[:], mask16[:])

        # ---------------- pass B: out = W.T @ Xh; y = out * x1 --------------
        for w in range(ws):
            x1t = x1s.tile([P, FS], F32)
            x1e = nc.scalar if w % 2 == 0 else nc.sync
            x1e.dma_start(out=x1t[:], in_=x1v[:, :, w, :])
            yt = ys.tile([P, FS], F32)
            for j in range(mpt):
                sl = slice(j * FM, (j + 1) * FM)
                op = ops.tile([PK, FM], F32, tag="op")
                nc.tensor.matmul(op[0:P, :], lhsT=wt[:], rhs=xh[w][:, sl],
                                 start=True, stop=True)
                nc.vector.tensor_mul(yt[:, sl], op[0:P, :], x1t[:, sl])
            # stores ride the gpsimd queue (idle after pass A): a store
            # waiting on its muls would FIFO-block the dependency-free x1
            # loads behind it on the HWDGE queues.  The last few stores go
            # back to HWDGE (no loads left behind them) to halve the drain.
            if w < ws - 4:
                nc.gpsimd.dma_start(out=yv[:, :, w, :], in_=yt[:])
            else:
                ye = (nc.sync, nc.scalar)[w % 2]
                ye.dma_start(out=yv[:, :, w, :], in_=yt[:])

    nc.compile()
    return nc


_NC_CACHE = {}


def _get_nc(d_total=D_FULL):
    if d_total not in _NC_CACHE:
        _NC_CACHE[d_total] = build_nc(d_total)
    return _NC_CACHE[d_total]


def kernel(x1: np.ndarray, x2: np.ndarray, gamma: np.ndarray) -> np.ndarray:
    b, n, c, h, w = x1.shape
    assert (b, n) == (B, NN)
    d = c * h * w
    x1r = np.ascontiguousarray(x1.reshape(b, n, d)).astype(np.float32, copy=False)
    x2r = np.ascontiguousarray(x2.reshape(b, n, d)).astype(np.float32, copy=False)
    g = np.asarray(gamma, dtype=np.float32).reshape(1)

    nc = _get_nc(d)
    in_maps = [
        {"x1": x1r[i], "x2": x2r[i], "gamma": g} for i in range(N_CORES)
    ]
    res = run_bass_kernel_spmd(nc, in_maps, list(range(N_CORES)))
    out = np.stack([res.results[i]["y"] for i in range(N_CORES)], axis=0)
    return out.reshape(b, n, c, h, w).astype(np.float32, copy=False)


# revision 42
# speedup vs baseline: 1.0653x; 1.0653x over previous
"""Trainium2 Bass kernel for CSAM channel self-attention module.

Per batch b (one per NeuronCore, B=8 over 8 cores):
    v      = x2[b].reshape(7, D)                 # D = 64*128*128 = 1048576
    E      = v @ v.T                             # [7,7] gram ("energy")
    att    = softmax(rowmax(E) - E, axis=-1)     # == exp(rowmin(E)-E)/Z
    out    = att @ v
    y[b]   = x1[b] * (gamma*out) + x1[b] = x1[b] * (gamma*out + 1)

Layout: d = q*65536 + w*2048 + f  (Q=16 runs, tiles [112, 2048] with
partition p = 7*q + m and 8KB contiguous DRAM lines).

Pass A: stream x2 via gpsimd SWDGE cast-DMAs (fp32 HBM -> fp16 SBUF cache
xh[w], one 917KB transfer per tile, no staging or DVE casts).  PE-transpose
[112,128] chunks -> PSUM [128,112] fp16, copy to SBUF (DVE/ACT alternating),
gram-matmul accumulate into E_psum[112,112] (diag 7x7 blocks = per-q partial
gram).  Gram matmuls trail the transposes by a few chunks so the in-order PE
queue never stalls on the copy round-trip.  fp16 is safe: top-2 energy gaps
are >100 while fp16 gram error is ~+-2.

Middle (all on-chip, ~2us): e_m = E_psum . blockdiag-mask (DVE), then
E7 = R1^T e_m R1 via two tiny PE matmuls (R1 = 16-stacked I7 folds the 16
diagonal blocks; cross-q junk is masked off first).  Softmax on-chip, then
W = mask16 . (R1x^T (a16 R1y)) -- two more tiny PE matmuls replicate
gamma*att into all 16 diagonal blocks, DVE mask keeps only those.  The
R1/R1x/R1y/mask constants are built once at startup via a DRAM bounce of I7
(engine ops cannot start at partition offsets like 7q or 112 -- only DMA can
touch those).  x1 prefetch (5 tiles on the idle HWDGE queues) covers the
middle's DMA gap.

Pass B: out_psum[112,512] = W.T @ Xh slices (fp16, "+1" fused via ones row
112, written by tiny gpsimd DMAs interleaved into the cast stream), y =
out_psum * x1 on DVE into a separate y-staging tile (an in-place variant
couples the x1-load to the y-store of the tile 8 back and strangles the
pipeline ramp -- measured 40us slower).  x2 is read from HBM exactly once.
"""

import sys

import numpy as np

try:
    import concourse.bass as bass
except ImportError:  # grading env fallback
    sys.path.insert(0, "/opt/trn_rl_repo")
    import concourse.bass as bass

from contextlib import ExitStack

import concourse.bacc as bacc
import concourse.tile as tile
from concourse import mybir
from concourse.bass_utils import run_bass_kernel_spmd
from concourse.masks import make_identity

F32 = mybir.dt.float32
F16 = mybir.dt.float16

B = 8
NN = 7              # attention dim
Q = 16              # d-runs per channel
P = NN * Q          # 112 partitions of (q, m)
PK = P + 1          # PSUM op-tile partition headroom (tile shape reuse)
FS = 2048           # stream tile free dim (8KB DRAM lines)
FM = 512            # matmul slice free dim (one PSUM bank)
D_FULL = 64 * 128 * 128
N_CORES = 8
PIPE = 8            # gram matmul trails transposes by this many chunks
GRP = 4             # transpose chunks batched per PSUM bank
X1PRE = 5           # x1 prefetch depth


def build_nc(d_total=D_FULL):
    assert d_total % (Q * FS) == 0
    ws = d_total // (Q * FS)          # stream tiles (32 at full size)
    cpt = FS // 128                   # transpose chunks per stream tile (16)
    mpt = FS // FM                    # matmul slices per stream tile (4)
    n_gram = ws * cpt

    nc = bacc.Bacc("TRN2", target_bir_lowering=False, debug=False)
    x1 = nc.dram_tensor("x1", [NN, d_total], F32, kind="ExternalInput")
    x2 = nc.dram_tensor("x2", [NN, d_total], F32, kind="ExternalInput")
    gm = nc.dram_tensor("gamma", [1], F32, kind="ExternalInput")
    y = nc.dram_tensor("y", [NN, d_total], F32, kind="ExternalOutput")

    x2v = x2[:].rearrange("m (q w f) -> q m w f", q=Q, w=ws, f=FS)
    x1v = x1[:].rearrange("m (q w f) -> q m w f", q=Q, w=ws, f=FS)
    yv = y[:].rearrange("m (q w f) -> q m w f", q=Q, w=ws, f=FS)

    with tile.TileContext(nc) as tc, ExitStack() as ctx:
        consts = ctx.enter_context(tc.tile_pool(name="consts", bufs=1))
        cache = ctx.enter_context(tc.tile_pool(name="cache", bufs=1))
        x1s = ctx.enter_context(tc.tile_pool(name="x1s", bufs=X1PRE))
        tsb = ctx.enter_context(tc.tile_pool(name="tsb", bufs=4))
        ys = ctx.enter_context(tc.tile_pool(name="ys", bufs=3))
        small = ctx.enter_context(tc.tile_pool(name="small", bufs=1))
        dramp = ctx.enter_context(tc.tile_pool(name="dramp", bufs=1, space="DRAM"))
        tps = ctx.enter_context(tc.tile_pool(name="tps", bufs=5, space="PSUM"))
        eps = ctx.enter_context(tc.tile_pool(name="eps", bufs=1, space="PSUM"))
        ops = ctx.enter_context(tc.tile_pool(name="ops", bufs=2, space="PSUM"))

        xh = [cache.tile([PK, FS], F16, name=f"xh{w}", tag=f"xh{w}")
              for w in range(ws)]
        wt = small.tile([PK, P], F16)

        # The fused-"+1" ones rows (row 112 of each xh tile and of wt) are
        # written by tiny gpsimd DMAs from a DRAM ones scratch, interleaved
        # into the cast-DMA stream AFTER each tile's cast (ranges are
        # disjoint, so order doesn't matter for the data -- but ones-first
        # made every cast wait a ones semaphore, denting the stream).  (An
        # engine memset cannot start at partition 112, and a [*,2048] memset
        # costs 1.8us of free-dim-serial engine time regardless of partition
        # count -- 32 of them at the DVE queue head stalled the whole pass.)
        ones16 = consts.tile([1, FS], F16)
        nc.vector.memset(ones16[:], 1.0)
        onescr = dramp.tile([1, FS], F16)
        nc.sync.dma_start(out=onescr[:], in_=ones16[:])

        # ---- gpsimd queue head: gamma, then the x2 cast stream ------------
        gsb = small.tile([NN, 1], F32)
        nc.gpsimd.dma_start(
            out=gsb[:],
            in_=bass.AP(tensor=gm[:].tensor, offset=0, ap=[[0, NN], [1, 1]]),
        )
        for w in range(2):
            nc.gpsimd.dma_start(out=xh[w][0:P, :], in_=x2v[:, :, w, :])

        ident = consts.tile([P, P], F16)
        make_identity(nc, ident)                       # gpsimd memset+select

        for w in range(2, ws):
            nc.gpsimd.dma_start(out=xh[w][0:P, :], in_=x2v[:, :, w, :])
            nc.gpsimd.dma_start(out=xh[w - 2][P:PK, :], in_=onescr[:])
        for w in range(ws - 2, ws):
            nc.gpsimd.dma_start(out=xh[w][P:PK, :], in_=onescr[:])
        nc.gpsimd.dma_start(out=wt[P:PK, :], in_=onescr[0:1, 0:P])

        # ---- remaining constants (off critical path) ----------------------

        # Fold/replicate operators from a DRAM bounce of I7:
        #   R1  [112,7]: R1[7q+m, n] = I7[m, n]
        #   R1r [7,112]: R1r[n, 7q+n'] = I7[n, n']
        id7_32 = consts.tile([NN, NN], F32)
        make_identity(nc, id7_32)
        id7_16 = consts.tile([NN, NN], F16)
        nc.vector.tensor_copy(out=id7_16[:], in_=id7_32[:])
        dscr = dramp.tile([NN, NN], F32)
        dscr16 = dramp.tile([NN, NN], F16)
        nc.scalar.dma_start(out=dscr[:], in_=id7_32[:])
        nc.scalar.dma_start(out=dscr16[:], in_=id7_16[:])
        r1_32 = consts.tile([P, NN], F32)
        nc.scalar.dma_start(
            out=r1_32[:],
            in_=bass.AP(tensor=dscr.tensor, offset=dscr.offset,
                        ap=[[0, Q], [NN, NN], [1, NN]]),
        )
        r1r_16 = consts.tile([NN, P], F16)
        r1r_dst = bass.AP(tensor=r1r_16.tensor, offset=r1r_16.offset,
                          ap=[[r1r_16.ap[0][0], NN], [NN, Q], [1, NN]])
        nc.scalar.dma_start(
            out=r1r_dst,
            in_=bass.AP(tensor=dscr16.tensor, offset=dscr16.offset,
                        ap=[[NN, NN], [0, Q], [1, NN]]),
        )
        # block-diag masks (built with tiny DMAs: engine writes cannot start
        # at partition 7q)
        ones7_32 = consts.tile([NN, NN], F32)
        nc.vector.memset(ones7_32[:], 1.0)
        ones7_16 = consts.tile([NN, NN], F16)
        nc.vector.memset(ones7_16[:], 1.0)
        mask32 = consts.tile([P, P], F32)
        nc.vector.memset(mask32[:], 0.0)
        mask16 = consts.tile([P, P], F16)
        nc.vector.memset(mask16[:], 0.0)
        for q in range(Q):
            s = slice(q * NN, (q + 1) * NN)
            nc.sync.dma_start(out=mask32[s, s], in_=ones7_32[:])
            nc.scalar.dma_start(out=mask16[s, s], in_=ones7_16[:])

        E = eps.tile([P, P], F32)

        # ~4us of dummy matmuls so the PE HAM clock-gate opens before the
        # real pass-A stream arrives (and stays open)
        for _ in range(24):
            wm = ops.tile([PK, FM], F32, tag="op")
            nc.tensor.matmul(wm[0:P, 0:P], lhsT=ident[:], rhs=ident[:],
                             start=True, stop=True)

        # ---------------- pass A: transpose + gram on the fp16 cache --------
        pend = []          # tt chunk APs awaiting gram matmul
        gi = 0             # gram matmuls emitted

        def emit_gram(tt_ap):
            nonlocal gi
            nc.tensor.matmul(E[:], lhsT=tt_ap, rhs=tt_ap,
                             start=(gi == 0), stop=(gi == n_gram - 1))
            gi += 1

        for w in range(ws):
            for g in range(cpt // GRP):
                tp = tps.tile([128, GRP * P], F16)
                for k in range(GRP):
                    c = g * GRP + k
                    nc.tensor.transpose(
                        tp[:, k * P:(k + 1) * P],
                        xh[w][0:P, c * 128:(c + 1) * 128], ident[:])
                tt = tsb.tile([128, GRP * P], F16)
                if (w * (cpt // GRP) + g) % 4 == 0:
                    nc.scalar.copy(tt[:], tp[:])
                else:
                    nc.vector.tensor_copy(out=tt[:], in_=tp[:])
                for k in range(GRP):
                    pend.append(tt[:, k * P:(k + 1) * P])
                while len(pend) > PIPE:
                    emit_gram(pend.pop(0))
        for tt in pend:
            emit_gram(tt)
        pend = []

        # ---------------- energy -> attention -> weights (on-chip) ---------
        e_m = small.tile([P, P], F32)
        nc.vector.tensor_mul(e_m[:], E[:], mask32[:])  # PSUM read + mask
        t1p = ops.tile([PK, FM], F32, tag="op")
        nc.tensor.matmul(t1p[0:P, 0:NN], lhsT=e_m[:], rhs=r1_32[:],
                         start=True, stop=True)        # fold n over q
        t1 = small.tile([P, NN], F32)
        nc.scalar.copy(t1[:], t1p[0:P, 0:NN])
        e7p = ops.tile([PK, FM], F32, tag="op")
        nc.tensor.matmul(e7p[0:NN, 0:NN], lhsT=r1_32[:], rhs=t1[:],
                         start=True, stop=True)        # fold m over q
        mn = small.tile([NN, 1], F32)
        nc.vector.tensor_reduce(
            out=mn[:], in_=e7p[0:NN, 0:NN], axis=mybir.AxisListType.X,
            op=mybir.AluOpType.min,
        )
        ex = small.tile([NN, NN], F32)
        nc.scalar.activation(
            out=ex[:], in_=e7p[0:NN, 0:NN],
            func=mybir.ActivationFunctionType.Exp,
            bias=mn[:], scale=-1.0,
        )                                              # exp(rowmin - E)
        z = small.tile([NN, 1], F32)
        nc.vector.tensor_reduce(
            out=z[:], in_=ex[:], axis=mybir.AxisListType.X,
            op=mybir.AluOpType.add,
        )
        r = small.tile([NN, 1], F32)
        nc.vector.reciprocal(r[:], z[:])
        rg = small.tile([NN, 1], F32)
        nc.vector.tensor_mul(rg[:], r[:], gsb[:])      # gamma / Z_n
        a16 = small.tile([NN, NN], F16)
        nc.vector.tensor_scalar_mul(a16[:], ex[:], rg[:])   # gamma*att, fp16
        arp = ops.tile([PK, FM], F32, tag="op")
        nc.tensor.matmul(arp[0:NN, 0:P], lhsT=a16[:], rhs=r1r_16[:],
                         start=True, stop=True)        # (g*att)^T tiled 16x
        arep = small.tile([NN, P], F16)
        nc.scalar.copy(arep[:], arp[0:NN, 0:P])
        wp = ops.tile([PK, FM], F32, tag="op")
        nc.tensor.matmul(wp[0:P, 0:P], lhsT=r1r_16[:], rhs=arep[:],
                         start=True, stop=True)        # replicate over q rows
        nc.vector.tensor_mul(wt[0:P, :], wp[0:P, 0:P], mask32[:])

        # ---------------- pass B: out = W.T @ Xh; y = out * x1 --------------
        for w in range(ws):
            x1t = x1s.tile([P, FS], F32)
            x1e = nc.scalar if w % 2 == 0 else nc.sync
            x1e.dma_start(out=x1t[:], in_=x1v[:, :, w, :])
            yt = ys.tile([P, FS], F32)
            for j in range(mpt):
                sl = slice(j * FM, (j + 1) * FM)
                op = ops.tile([PK, FM], F32, tag="op")
                nc.tensor.matmul(op[0:P, :], lhsT=wt[:], rhs=xh[w][:, sl],
                                 start=True, stop=True)
                nc.vector.tensor_mul(yt[:, sl], op[0:P, :], x1t[:, sl])
            # stores ride the gpsimd queue (idle after pass A): a store
            # waiting on its muls would FIFO-block the dependency-free x1
            # loads behind it on the HWDGE queues.  The last few stores go
            # back to HWDGE (no loads left behind them) to halve the drain.
            if w < ws - 4:
                nc.gpsimd.dma_start(out=yv[:, :, w, :], in_=yt[:])
            else:
                ye = (nc.sync, nc.scalar)[w % 2]
                ye.dma_start(out=yv[:, :, w, :], in_=yt[:])

    nc.compile()
    return nc


_NC_CACHE = {}


def _get_nc(d_total=D_FULL):
    if d_total not in _NC_CACHE:
        _NC_CACHE[d_total] = build_nc(d_total)
    return _NC_CACHE[d_total]


def kernel(x1: np.ndarray, x2: np.ndarray, gamma: np.ndarray) -> np.ndarray:
    b, n, c, h, w = x1.shape
    assert (b, n) == (B, NN)
    d = c * h * w
    x1r = np.ascontiguousarray(x1.reshape(b, n, d)).astype(np.float32, copy=False)
    x2r = np.ascontiguousarray(x2.reshape(b, n, d)).astype(np.float32, copy=False)
    g = np.asarray(gamma, dtype=np.float32).reshape(1)

    nc = _get_nc(d)
    in_maps = [
        {"x1": x1r[i], "x2": x2r[i], "gamma": g} for i in range(N_CORES)
    ]
    res = run_bass_kernel_spmd(nc, in_maps, list(range(N_CORES)))
    out = np.stack([res.results[i]["y"] for i in range(N_CORES)], axis=0)
    return out.reshape(b, n, c, h, w).astype(np.float32, copy=False)
